# revision 9
# baseline (speedup 1.0000x reference)
"""Trainium2 Bass kernel for nn_BidPrefix (segment_reduce).

Problem: inputs [B=500000, 302] f32 rows = [rates[0:300], market_price, bid].
  cp1[k] = prod(rates[:k])  (exclusive prefix products, cp1[0] = 1)
  survival  = cp1[bid]
  rate_last = cp1[mp] - cp1[mp+1]

Kernel strategy (pure data parallel over 8 NeuronCores, batch sharded):
  Log-domain masked sums spread across three engines per 128-row tile:
    - one batched DMA (3 tiles per dma_start) loads [128, 3*302] f32;
    - ACT computes lr = ln(rates) fp32 -> fp16 for the whole 3-tile
      batch in one pass (otherwise idle engine; batching amortizes the
      ACT per-op SBUF-latency overhead);
    - DVE runs three fp16 scalar_tensor_tensor masked sums with fused
      accumulate: S += (iota < bid)*lr, A1 += (iota < mp)*lr,
      A2 += (iota <= mp)*lr  (fp16 operands enable the faster DVE perf
      modes; the fp32 scalar/accum operands are exempt from the dtype
      rule).
  Tail: exp() on the three [128, ntiles] accumulators (ACT), one wide
  subtract, and two partition-contiguous output DMAs; the host undoes
  the (p, t) -> t*128+p permutation.

This removes the serial cumprod scan entirely (exp(sum(log)) == prod)
and cuts DVE work from 4 fp32 passes/tile to 2 fp16 passes/tile.
"""

import numpy as np

SEQ = 300
W = SEQ + 2  # input columns
B = 500000
N_CORES = 8
ROWS_PER_CORE = 62592  # 489 tiles of 128 rows; 8*62592 = 500736 >= B
NTILES = ROWS_PER_CORE // 128
DMA_BATCH = 3  # tiles per input dma_start; 489 = 3*163
assert ROWS_PER_CORE % 128 == 0 and N_CORES * ROWS_PER_CORE >= B
assert NTILES % DMA_BATCH == 0

_CACHE = {}


def _split_multi_waits(nc, max_waits=1):
    """Walrus in this container rejects instructions with >1 sync-wait.

    Hoist extra waits onto single-wait NOPs inserted right before the
    offending instruction on the same engine (same-queue program order
    preserves semantics).
    """
    import concourse.mybir as mybir

    ctr = 0
    for fn in nc.m.functions:
        for bb in fn.blocks:
            il = bb.instructions
            i = 0
            while i < len(il):
                ins = il[i]
                si = ins.sync_info
                if si is not None and si.on_wait and len(si.on_wait) > max_waits:
                    waits = list(si.on_wait)
                    pos = i
                    for w in waits[max_waits:]:
                        ctr += 1
                        nop = mybir.InstNoOp(
                            name=f"I-splitwait-{ctr}",
                            engine=ins.engine,
                            sync_info=mybir.SyncInfo(on_wait=[w], on_update=[]),
                        )
                        il.insert(pos, nop)
                        pos += 1
                        i += 1
                    si.on_wait = waits[:max_waits]
                i += 1


def _build_nc(rows=ROWS_PER_CORE, in_bufs=8, lr_bufs=8, tr_bufs=6):
    import concourse.bass as bass
    import concourse.tile as tile
    from concourse import mybir

    F32 = mybir.dt.float32
    F16 = mybir.dt.float16
    I32 = mybir.dt.int32
    assert rows % 128 == 0
    ntiles = rows // 128

    nc = bass.Bass("TRN2")
    x = nc.dram_tensor("inputs", [rows, W], F32, kind="ExternalInput")
    out_s = nc.dram_tensor("surv", [rows, 1], F32, kind="ExternalOutput")
    out_r = nc.dram_tensor("ratelast", [rows, 1], F32, kind="ExternalOutput")

    # Batched loads: one DMA brings DMA_BATCH tiles; sbuf[p, t, c] =
    # dram[b*DMA_BATCH*128 + t*128 + p, c].
    x_b = x.rearrange("(b t p) c -> b p t c", t=DMA_BATCH, p=128)
    # Partition-contiguous stores: dram row = p*ntiles + t (host unpermutes).
    out_s_t = out_s.rearrange("(p t) c -> p t c", p=128)
    out_r_t = out_r.rearrange("(p t) c -> p t c", p=128)

    Ln = mybir.ActivationFunctionType.Ln
    Exp = mybir.ActivationFunctionType.Exp

    with tile.TileContext(nc) as tc:
        with (
            tc.tile_pool(name="inp", bufs=in_bufs) as inp_pool,
            tc.tile_pool(name="lr", bufs=lr_bufs) as lr_pool,
            tc.tile_pool(name="trv", bufs=tr_bufs) as trv_pool,
            tc.tile_pool(name="trp", bufs=tr_bufs) as trp_pool,
            tc.tile_pool(name="persist", bufs=1) as persist,
        ):
            S = persist.tile([128, ntiles], F32, tag="acc_s")
            A1 = persist.tile([128, ntiles], F32, tag="acc_a1")
            A2 = persist.tile([128, ntiles], F32, tag="acc_a2")

            iota_i = persist.tile([128, SEQ], I32, tag="iota_i")
            nc.gpsimd.iota(iota_i[:, :], [[1, SEQ]], channel_multiplier=0)
            iota_h = persist.tile([128, SEQ], F16, tag="iota_h")
            nc.vector.tensor_copy(iota_h[:, :], iota_i[:, :])

            for i in range(ntiles):
                bi, ti = divmod(i, DMA_BATCH)
                if ti == 0:
                    xb = inp_pool.tile([128, DMA_BATCH, W], F32, tag="xb")
                    nc.sync.dma_start(out=xb[:, :, :], in_=x_b[bi, :, :, :])
                    # One ln pass for the whole 3-tile batch amortizes the
                    # ACT per-op SBUF-latency overhead.
                    lrb = lr_pool.tile([128, DMA_BATCH, SEQ], F16, tag="lr")
                    nc.scalar.activation(lrb[:, :, :], xb[:, :, 0:SEQ], Ln)

                xt = xb[:, ti, :]
                mp = xt[:, SEQ : SEQ + 1]
                bid = xt[:, SEQ + 1 : SEQ + 2]

                for eng, idx_ap, op0, acc, pool in (
                    (nc.vector, bid, mybir.AluOpType.is_lt, S, trv_pool),
                    (nc.vector, mp, mybir.AluOpType.is_lt, A1, trv_pool),
                    (nc.vector, mp, mybir.AluOpType.is_le, A2, trp_pool),
                ):
                    tr = pool.tile([128, SEQ], F16, tag="tr")
                    eng.scalar_tensor_tensor(
                        out=tr[:, :],
                        in0=iota_h[:, :],
                        scalar=idx_ap,
                        in1=lrb[:, ti, :],
                        op0=op0,
                        op1=mybir.AluOpType.mult,
                        accum_out=acc[:, i : i + 1],
                    )

            ES = persist.tile([128, ntiles], F32, tag="es")
            E1 = persist.tile([128, ntiles], F32, tag="e1")
            E2 = persist.tile([128, ntiles], F32, tag="e2")
            nc.scalar.activation(ES[:, :], S[:, :], Exp)
            nc.scalar.activation(E1[:, :], A1[:, :], Exp)
            nc.scalar.activation(E2[:, :], A2[:, :], Exp)
            nc.vector.tensor_sub(E1[:, :], E1[:, :], E2[:, :])
            nc.sync.dma_start(out=out_s_t[:, :, 0], in_=ES[:, :])
            nc.sync.dma_start(out=out_r_t[:, :, 0], in_=E1[:, :])

    _split_multi_waits(nc)
    return nc


def _get_nc():
    if "nc" not in _CACHE:
        _CACHE["nc"] = _build_nc()
    return _CACHE["nc"]


def _shard_inputs(inputs):
    total = N_CORES * ROWS_PER_CORE
    padded = np.empty((total, W), dtype=np.float32)
    padded[: inputs.shape[0]] = inputs
    if total > inputs.shape[0]:
        padded[inputs.shape[0] :, :SEQ] = 1.0
        padded[inputs.shape[0] :, SEQ:] = 0.0
    return [
        padded[c * ROWS_PER_CORE : (c + 1) * ROWS_PER_CORE] for c in range(N_CORES)
    ]


def _unpermute(col):
    # DRAM row index is p*NTILES + t; original tile row is t*128 + p.
    return col.reshape(128, NTILES).T.reshape(ROWS_PER_CORE, 1)


def kernel(inputs: np.ndarray):
    from concourse.bass_utils import run_bass_kernel_spmd

    inputs = np.ascontiguousarray(inputs, dtype=np.float32)
    assert inputs.shape == (B, W), inputs.shape

    nc = _get_nc()
    shards = _shard_inputs(inputs)
    res = run_bass_kernel_spmd(
        nc,
        [{"inputs": s} for s in shards],
        core_ids=list(range(N_CORES)),
    )
    surv = np.concatenate([_unpermute(r["surv"]) for r in res.results], axis=0)[:B]
    rl = np.concatenate([_unpermute(r["ratelast"]) for r in res.results], axis=0)[:B]
    return surv, rl
